# revision 4
# baseline (speedup 1.0000x reference)
"""Trainium2 Bass kernel for nn_AttentionLayer (additive attention pooling).

reference math:
    re = entities @ w1_w + w1_b                  # [B, H]
    rc = contexts @ w2_w + w2_b                  # [B, S, H]
    scores = tanh(re[:,None,:] + rc) @ v_w + v_b # [B, S, 1]
    weights = softmax(scores, axis=1)
    out = weights * contexts                     # [B, S, D]

Sharding: data-parallel over B across 8 cores (4 batches/core), weights
replicated.  Inside each core: bf16 TensorEngine matmuls (f32 accumulate),
softmax in f32/f16.  v_b is dropped (softmax is shift-invariant).

v2 dataflow (per core):
  - contexts/weights stream in via SWDGE (gpsimd) DMAs that cast f32->bf16
    in flight; interleaved so the first main matmul fires ~10us in.
  - per 512-token chunk: one xbar DMA transpose (sync ring) makes the
    d-major rhs; 64 accumulating bf16 matmuls produce rc; tanh(+re bias)
    on ACT; a v-matvec accumulates chunk scores in PSUM.
  - matvec for (chunk,ho) is emitted one ho-group later than its tanh so
    the PE never waits on ACT.
  - chunk scores [1,512] are copied to fp16 and transposed token-major via
    4 tiny PE matmuls into a per-batch [128,16] PSUM tile; softmax runs
    full-width (exp+accum on ACT, cross-partition total via a ones-matmul,
    reciprocal on DVE) - no single-lane work.
  - out tiles = bf16 contexts * per-token weight (DVE), stored via the
    scalar-engine HWDGE ring so stores never queue behind transposes.
"""

import sys

for _p in ("/opt/trn_rl_repo", "/root/.axon_site/_ro/trn_rl_repo"):
    if _p not in sys.path:
        sys.path.insert(0, _p)

import numpy as np

B, S, D, H = 32, 2048, 1024, 1024
N_CORES = 8
B_LOC = B // N_CORES          # batches per core
P = 128
TCHUNK = 512                  # tokens per chunk (moving free dim of main matmul)


def build_attention(tc, out_ap, ins, b_loc=B_LOC, s=S, d=D, h=H):
    """Emit the per-core kernel into TileContext `tc`.

    out_ap: DRAM AP [b_loc*s, d] f32
    ins: dict of DRAM APs: contexts [b_loc*s, d], entities [b_loc, d],
         w1_w [d, h], w2_w [d, h], w1_b [h], w2_b [h], v_w [h, 1]
    """
    from contextlib import ExitStack

    import concourse.mybir as mybir
    from concourse.masks import make_identity

    nc = tc.nc
    f32 = mybir.dt.float32
    bf16 = mybir.dt.bfloat16
    f16 = mybir.dt.float16
    AF = mybir.ActivationFunctionType

    KO = d // P                   # contraction k-tiles
    HO = h // P                   # h tiles
    NT = s // P                   # 128-token tiles per batch
    NC = s // TCHUNK              # chunks per batch
    TPC = TCHUNK // P             # token tiles per chunk
    QW = 256                      # h-chunk width for weight staging DMAs
    NQ = h // QW
    EP = 32                       # padded partition count for entity transposes
    assert d % P == 0 and h % P == 0 and s % TCHUNK == 0 and b_loc <= EP

    ctx3 = ins["contexts"].rearrange("(n p) dd -> n p dd", p=P)   # [b_loc*NT, P, d]
    out3 = out_ap.rearrange("(n p) dd -> n p dd", p=P)
    w1_3d = ins["w1_w"].rearrange("(ko p) hh -> p ko hh", p=P)
    w2_3d = ins["w2_w"].rearrange("(ko p) hh -> p ko hh", p=P)

    with ExitStack() as ctx:
        consts = ctx.enter_context(tc.tile_pool(name="consts", bufs=1))
        wpool = ctx.enter_context(tc.tile_pool(name="wpool", bufs=1))

        # ---------------- constants (tiny, sync ring) ----------------
        id32 = consts.tile([EP, EP], f32, tag="id32")
        make_identity(nc, id32)
        ones1_f16 = consts.tile([1, 1], f16, tag="ones1")
        nc.vector.memset(ones1_f16, 1.0)
        ones128_f16 = consts.tile([P, P], f16, tag="ones128")
        nc.vector.memset(ones128_f16, 1.0)

        ent_sb = consts.tile([EP, d], f32, tag="ent")
        nc.vector.memset(ent_sb, 0.0)
        nc.sync.dma_start(out=ent_sb[:b_loc, :], in_=ins["entities"][:, :])

        b1_sb = consts.tile([P, HO], f32, tag="b1")
        b2_sb = consts.tile([P, HO], f32, tag="b2")
        nc.sync.dma_start(out=b1_sb, in_=ins["w1_b"].rearrange("(ho p) -> p ho", p=P))
        nc.sync.dma_start(out=b2_sb, in_=ins["w2_b"].rearrange("(ho p) -> p ho", p=P))
        bias_sb = consts.tile([P, HO], f32, tag="bias")
        nc.vector.tensor_add(out=bias_sb, in0=b1_sb, in1=b2_sb)

        # ---------------- weight / context tiles ----------------
        w1_bf = wpool.tile([P, KO, h], bf16, tag="w1bf")
        w2_bf = wpool.tile([P, KO, h], bf16, tag="w2bf")
        v_bf = consts.tile([P, HO, 1], bf16, tag="v_bf")

        xbf_pool = ctx.enter_context(tc.tile_pool(name="xbf", bufs=8))
        xt_pool = ctx.enter_context(tc.tile_pool(name="xt", bufs=3))
        th_pool = ctx.enter_context(tc.tile_pool(name="th", bufs=4))
        out_pool = ctx.enter_context(tc.tile_pool(name="outp", bufs=4))
        sm_pool = ctx.enter_context(tc.tile_pool(name="smx", bufs=2))

        def load_w(dst3, src3, q):
            # SWDGE cast f32 -> bf16 during the DMA
            nc.gpsimd.dma_start(
                out=dst3[:, :, q * QW : (q + 1) * QW],
                in_=src3[:, :, q * QW : (q + 1) * QW],
            )

        xbf_tiles = {}

        def load_ctx(b, c):
            xc = xbf_pool.tile([P, TPC, d], bf16, tag="xbf")
            r0 = b * NT + c * TPC
            nc.gpsimd.dma_start(
                out=xc, in_=ctx3[r0 : r0 + TPC].rearrange("n p dd -> p n dd")
            )
            xbf_tiles[(b, c)] = xc

        # ---------------- interleaved preamble loads (gpsimd ring) ----------------
        # w1 first chunk feeds the entity path; contexts(b0,c0) + w2 low
        # h-chunks feed the first main matmuls; the rest stream behind.
        load_w(w1_bf, w1_3d, 0)
        load_ctx(0, 0)
        nc.gpsimd.dma_start(
            out=v_bf, in_=ins["v_w"].rearrange("(ho p) o -> p ho o", p=P)
        )
        load_w(w2_bf, w2_3d, 0)
        load_w(w2_bf, w2_3d, 1)
        load_ctx(0, 1)
        load_w(w2_bf, w2_3d, 2)
        load_w(w1_bf, w1_3d, 1)
        load_ctx(0, 2)
        load_w(w2_bf, w2_3d, 3)
        load_w(w1_bf, w1_3d, 2)
        load_w(w1_bf, w1_3d, 3)
        load_ctx(0, 3)

        # ---------------- entity path: reb[:, ho, b] = (entities@w1 + b1+b2)^T ----
        reb_sb = consts.tile([P, HO, b_loc], f32, tag="reb")
        with tc.tile_pool(name="ps_pre", bufs=2, space="PSUM") as ps_pre:
            entT_bf = consts.tile([P, KO, b_loc], bf16, tag="entT")
            for ko in range(KO):
                etr = ps_pre.tile([P, EP], f32, tag="pre")
                nc.tensor.transpose(etr, ent_sb[:, ko * P : (ko + 1) * P], id32)
                nc.vector.tensor_copy(out=entT_bf[:, ko, :], in_=etr[:, :b_loc])

            re_sb = consts.tile([EP, h], f32, tag="re_sb")
            nc.vector.memset(re_sb, 0.0)
            for q in range(NQ):
                re_ps = ps_pre.tile([b_loc, QW], f32, tag="re")
                for ko in range(KO):
                    nc.tensor.matmul(
                        re_ps,
                        lhsT=entT_bf[:, ko, :],
                        rhs=w1_bf[:, ko, q * QW : (q + 1) * QW],
                        start=(ko == 0),
                        stop=(ko == KO - 1),
                    )
                nc.scalar.copy(out=re_sb[:b_loc, q * QW : (q + 1) * QW], in_=re_ps)
                for ho in range(q * QW // P, (q + 1) * QW // P):
                    rtr = ps_pre.tile([P, EP], f32, tag="pre")
                    nc.tensor.transpose(rtr, re_sb[:, ho * P : (ho + 1) * P], id32)
                    nc.vector.tensor_scalar(
                        out=reb_sb[:, ho, :],
                        in0=rtr[:, :b_loc],
                        scalar1=bias_sb[:, ho : ho + 1],
                        scalar2=None,
                        op0=mybir.AluOpType.add,
                    )

        # ---------------- main-loop PSUM pools ----------------
        ps_rc = ctx.enter_context(tc.tile_pool(name="ps_rc", bufs=4, space="PSUM"))
        ps_sc = ctx.enter_context(tc.tile_pool(name="ps_sc", bufs=2, space="PSUM"))
        ps_wt = ctx.enter_context(tc.tile_pool(name="ps_wt", bufs=2, space="PSUM"))

        # per-batch state shared between emission helpers
        state = {}

        def emit_matvec(b, T, ho):
            st = state[b]
            if ho == 0:
                st["sc"][T] = ps_sc.tile([1, TCHUNK], f32, tag="sc", name="sc_ps")
            nc.tensor.matmul(
                st["sc"][T],
                lhsT=v_bf[:, ho, :],
                rhs=st["th"].pop((T, ho)),
                start=(ho == 0),
                stop=(ho == HO - 1),
            )

        def emit_score_copy(b, T):
            st = state[b]
            swb = sm_pool.tile([1, TCHUNK], f16, tag="swb", bufs=3)
            nc.scalar.copy(out=swb, in_=st["sc"][T])
            st["swb"][T] = swb

        def emit_score_transpose(b, T):
            st = state[b]
            swb = st["swb"].pop(T)
            for j in range(TPC):
                cidx = T * TPC + j
                nc.tensor.matmul(
                    st["wt"][:, cidx : cidx + 1],
                    lhsT=swb[:, j * P : (j + 1) * P],
                    rhs=ones1_f16,
                    start=(T == 0 and j == 0),
                    stop=(T == NC - 1 and j == TPC - 1),
                )

        def emit_softmax(b):
            st = state[b]
            wt = st["wt"]
            ew = sm_pool.tile([P, NT], f16, tag="ew")
            asum = sm_pool.tile([P, 1], f32, tag="asum")
            nc.scalar.activation(
                out=ew, in_=wt[:, :NT], func=AF.Exp, accum_out=asum
            )
            asum16 = sm_pool.tile([P, 1], f16, tag="asum16")
            nc.vector.tensor_copy(out=asum16, in_=asum)
            # cross-partition total, broadcast to every partition via ones^T @ asum
            nc.tensor.matmul(
                wt[:, NT : NT + 1], lhsT=ones128_f16, rhs=asum16, start=True, stop=True
            )
            rb = sm_pool.tile([P, 1], f32, tag="rb")
            nc.vector.reciprocal(out=rb, in_=wt[:, NT : NT + 1])
            wts = sm_pool.tile([P, NT], f32, tag="wts")
            nc.vector.tensor_scalar_mul(out=wts, in0=ew, scalar1=rb)
            st["wts"] = wts

        def emit_stage_f(b):
            st = state[b]
            wts = st["wts"]
            last = b == b_loc - 1
            for t in range(NT):
                xc = xbf_tiles[(b, t // TPC)]
                src = xc[:, t % TPC, :]
                ot = out_pool.tile([P, d], f32, tag="ot")
                if last and t % 2 == 1:
                    # split the tail multiplies across ACT and DVE
                    nc.scalar.activation(
                        out=ot, in_=src, func=AF.Copy, scale=wts[:, t : t + 1]
                    )
                else:
                    nc.vector.tensor_scalar_mul(
                        out=ot, in0=src, scalar1=wts[:, t : t + 1]
                    )
                # stores ride the ACT HWDGE ring mid-kernel (sync ring is busy
                # with transposes); the tail batch uses the now-idle sync ring.
                eng = nc.sync if last else nc.scalar
                eng.dma_start(out=out3[b * NT + t], in_=ot)
            for c in range(NC):
                xbf_tiles.pop((b, c))

        # ---------------- main loop over local batches ----------------
        for b in range(b_loc):
            if b > 0:
                for c in range(NC):
                    load_ctx(b, c)
            state[b] = {"th": {}, "sc": {}, "swb": {}, "wt": None}
            state[b]["wt"] = ps_wt.tile([P, NT + 1], f32, tag="wt", name="wt_ps")

            for T in range(NC):
                xt = xt_pool.tile([P, TPC, KO, P], bf16, tag="xt")
                nc.sync.dma_start(
                    out=xt, in_=xbf_tiles[(b, T)], transpose=True
                )

                for ho in range(HO):
                    rc = ps_rc.tile([P, TCHUNK], f32, tag="rc")
                    for ko in range(KO):
                        nc.tensor.matmul(
                            rc,
                            lhsT=w2_bf[:, ko, ho * P : (ho + 1) * P],
                            rhs=xt[:, :, ko, :],
                            start=(ko == 0),
                            stop=(ko == KO - 1),
                        )
                    th = th_pool.tile([P, TCHUNK], bf16, tag="th")
                    nc.scalar.activation(
                        out=th,
                        in_=rc,
                        func=AF.Tanh,
                        bias=reb_sb[:, ho, b : b + 1],
                        scale=1.0,
                    )
                    state[b]["th"][(T, ho)] = th

                    # deferred PE work, staggered so it never waits on ACT
                    if ho >= 1:
                        emit_matvec(b, T, ho - 1)
                    if T >= 1:
                        if ho == 0:
                            emit_matvec(b, T - 1, HO - 1)
                            emit_score_copy(b, T - 1)
                        elif ho == 1:
                            emit_score_transpose(b, T - 1)
                    elif b >= 1:
                        # previous batch's tail rides this batch's first chunk
                        if ho == 0:
                            emit_matvec(b - 1, NC - 1, HO - 1)
                            emit_score_copy(b - 1, NC - 1)
                        elif ho == 1:
                            emit_score_transpose(b - 1, NC - 1)
                        elif ho == 2:
                            emit_softmax(b - 1)
                        elif ho == 3:
                            emit_stage_f(b - 1)
                            del state[b - 1]

        # tail: last batch's remaining score work + softmax + stage F
        bl = b_loc - 1
        emit_matvec(bl, NC - 1, HO - 1)
        emit_score_copy(bl, NC - 1)
        emit_score_transpose(bl, NC - 1)
        emit_softmax(bl)
        emit_stage_f(bl)


def build_module(b_loc=B_LOC, s=S, d=D, h=H):
    """Build and compile the Bacc module for one core (SPMD-replicated)."""
    import concourse.mybir as mybir
    import concourse.tile as tile
    from concourse import bacc

    f32 = mybir.dt.float32
    nc = bacc.Bacc("TRN2", target_bir_lowering=False, debug=False)

    ins = {
        "contexts": nc.dram_tensor("contexts", [b_loc * s, d], f32, kind="ExternalInput").ap(),
        "entities": nc.dram_tensor("entities", [b_loc, d], f32, kind="ExternalInput").ap(),
        "w1_w": nc.dram_tensor("w1_w", [d, h], f32, kind="ExternalInput").ap(),
        "w2_w": nc.dram_tensor("w2_w", [d, h], f32, kind="ExternalInput").ap(),
        "w1_b": nc.dram_tensor("w1_b", [h], f32, kind="ExternalInput").ap(),
        "w2_b": nc.dram_tensor("w2_b", [h], f32, kind="ExternalInput").ap(),
        "v_w": nc.dram_tensor("v_w", [h, 1], f32, kind="ExternalInput").ap(),
    }
    out_ap = nc.dram_tensor("out", [b_loc * s, d], f32, kind="ExternalOutput").ap()

    with tile.TileContext(nc) as tc:
        build_attention(tc, out_ap, ins, b_loc=b_loc, s=s, d=d, h=h)

    nc.compile()
    return nc


_NC_CACHE = {}


def _get_module():
    key = (B_LOC, S, D, H)
    if key not in _NC_CACHE:
        _NC_CACHE[key] = build_module(*key)
    return _NC_CACHE[key]


def make_in_maps(inputs):
    entities = np.ascontiguousarray(np.asarray(inputs["entities"], np.float32))
    contexts = np.ascontiguousarray(np.asarray(inputs["contexts"], np.float32))
    shared = {
        k: np.ascontiguousarray(np.asarray(inputs[k], np.float32))
        for k in ("w1_w", "w2_w", "w1_b", "w2_b", "v_w")
    }
    in_maps = []
    for c in range(N_CORES):
        in_maps.append(
            dict(
                entities=entities[c * B_LOC : (c + 1) * B_LOC],
                contexts=contexts[c * B_LOC : (c + 1) * B_LOC].reshape(B_LOC * S, D),
                **shared,
            )
        )
    return in_maps


def run(inputs, trace=False, **kwargs):
    """Run on all 8 cores; returns (full_output, BassKernelResults)."""
    from concourse.bass_utils import run_bass_kernel_spmd

    nc = _get_module()
    res = run_bass_kernel_spmd(
        nc, make_in_maps(inputs), core_ids=list(range(N_CORES)), trace=trace, **kwargs
    )
    out = np.concatenate(
        [res.results[c]["out"].reshape(B_LOC, S, D) for c in range(N_CORES)], axis=0
    )
    return out, res


def kernel(**inputs) -> np.ndarray:
    out, _ = run(inputs, trace=False)
    return out


# revision 6
# speedup vs baseline: 1.0154x; 1.0154x over previous
"""Trainium2 Bass kernel for nn_AttentionLayer (additive attention pooling).

reference math:
    re = entities @ w1_w + w1_b                  # [B, H]
    rc = contexts @ w2_w + w2_b                  # [B, S, H]
    scores = tanh(re[:,None,:] + rc) @ v_w + v_b # [B, S, 1]
    weights = softmax(scores, axis=1)
    out = weights * contexts                     # [B, S, D]

Sharding: data-parallel over B across 8 cores (4 batches/core), weights
replicated.  Inside each core: bf16 TensorEngine matmuls (f32 accumulate),
softmax in f32/f16.  v_b is dropped (softmax is shift-invariant).

v2 dataflow (per core):
  - contexts/weights stream in via SWDGE (gpsimd) DMAs that cast f32->bf16
    in flight; interleaved so the first main matmul fires ~10us in.
  - per 512-token chunk: one xbar DMA transpose (sync ring) makes the
    d-major rhs; 64 accumulating bf16 matmuls produce rc; tanh(+re bias)
    on ACT; a v-matvec accumulates chunk scores in PSUM.
  - matvec for (chunk,ho) is emitted one ho-group later than its tanh so
    the PE never waits on ACT.
  - chunk scores [1,512] are copied to fp16 and transposed token-major via
    4 tiny PE matmuls into a per-batch [128,16] PSUM tile; softmax runs
    full-width (exp+accum on ACT, cross-partition total via a ones-matmul,
    reciprocal on DVE) - no single-lane work.
  - out tiles = bf16 contexts * per-token weight (DVE), stored via the
    scalar-engine HWDGE ring so stores never queue behind transposes.
"""

import sys

for _p in ("/opt/trn_rl_repo", "/root/.axon_site/_ro/trn_rl_repo"):
    if _p not in sys.path:
        sys.path.insert(0, _p)

import numpy as np

B, S, D, H = 32, 2048, 1024, 1024
N_CORES = 8
B_LOC = B // N_CORES          # batches per core
P = 128
TCHUNK = 512                  # tokens per chunk (moving free dim of main matmul)


def build_attention(tc, out_ap, ins, b_loc=B_LOC, s=S, d=D, h=H):
    """Emit the per-core kernel into TileContext `tc`.

    out_ap: DRAM AP [b_loc*s, d] f32
    ins: dict of DRAM APs: contexts [b_loc*s, d], entities [b_loc, d],
         w1_w [d, h], w2_w [d, h], w1_b [h], w2_b [h], v_w [h, 1]
    """
    from contextlib import ExitStack

    import concourse.mybir as mybir
    from concourse.masks import make_identity

    nc = tc.nc
    f32 = mybir.dt.float32
    bf16 = mybir.dt.bfloat16
    f16 = mybir.dt.float16
    AF = mybir.ActivationFunctionType

    KO = d // P                   # contraction k-tiles
    HO = h // P                   # h tiles
    NT = s // P                   # 128-token tiles per batch
    NC = s // TCHUNK              # chunks per batch
    TPC = TCHUNK // P             # token tiles per chunk
    QW = 256                      # h-chunk width for weight staging DMAs
    NQ = h // QW
    EP = 32                       # padded partition count for entity transposes
    assert d % P == 0 and h % P == 0 and s % TCHUNK == 0 and b_loc <= EP

    ctx3 = ins["contexts"].rearrange("(n p) dd -> n p dd", p=P)   # [b_loc*NT, P, d]
    out3 = out_ap.rearrange("(n p) dd -> n p dd", p=P)
    w1_3d = ins["w1_w"].rearrange("(ko p) hh -> p ko hh", p=P)
    w2_3d = ins["w2_w"].rearrange("(ko p) hh -> p ko hh", p=P)

    with ExitStack() as ctx:
        consts = ctx.enter_context(tc.tile_pool(name="consts", bufs=1))
        wpool = ctx.enter_context(tc.tile_pool(name="wpool", bufs=1))

        # ---------------- constants (tiny, sync ring) ----------------
        id32 = consts.tile([EP, EP], f32, tag="id32")
        make_identity(nc, id32)
        ones1_f16 = consts.tile([1, 1], f16, tag="ones1")
        nc.vector.memset(ones1_f16, 1.0)
        ones128_f16 = consts.tile([P, P], f16, tag="ones128")
        nc.vector.memset(ones128_f16, 1.0)

        ent_sb = consts.tile([EP, d], f32, tag="ent")
        nc.vector.memset(ent_sb, 0.0)
        nc.sync.dma_start(out=ent_sb[:b_loc, :], in_=ins["entities"][:, :])

        b1_sb = consts.tile([P, HO], f32, tag="b1")
        b2_sb = consts.tile([P, HO], f32, tag="b2")
        nc.sync.dma_start(out=b1_sb, in_=ins["w1_b"].rearrange("(ho p) -> p ho", p=P))
        nc.sync.dma_start(out=b2_sb, in_=ins["w2_b"].rearrange("(ho p) -> p ho", p=P))
        bias_sb = consts.tile([P, HO], f32, tag="bias")
        nc.vector.tensor_add(out=bias_sb, in0=b1_sb, in1=b2_sb)

        # ---------------- weight / context tiles ----------------
        w1_bf = wpool.tile([P, KO, h], bf16, tag="w1bf")
        w2_bf = wpool.tile([P, KO, h], bf16, tag="w2bf")
        v_bf = consts.tile([P, HO, 1], bf16, tag="v_bf")
        v_st = consts.tile([P, HO, 1], f32, tag="v_st")

        xbf_pool = ctx.enter_context(tc.tile_pool(name="xbf", bufs=8))
        xt_pool = ctx.enter_context(tc.tile_pool(name="xt", bufs=3))
        th_pool = ctx.enter_context(tc.tile_pool(name="th", bufs=4))
        out_pool = ctx.enter_context(tc.tile_pool(name="outp", bufs=3))
        sm_pool = ctx.enter_context(tc.tile_pool(name="smx", bufs=2))
        cin_pool = ctx.enter_context(tc.tile_pool(name="cin", bufs=3))
        wst_pool = ctx.enter_context(tc.tile_pool(name="wst", bufs=2))

        def load_w(dst3, src3, q):
            # HWDGE f32 load + DVE cast to bf16
            wst = wst_pool.tile([P, KO, QW], f32, tag="wst")
            nc.sync.dma_start(out=wst, in_=src3[:, :, q * QW : (q + 1) * QW])
            nc.vector.tensor_copy(out=dst3[:, :, q * QW : (q + 1) * QW], in_=wst)

        xbf_tiles = {}
        xt_tiles = {}

        def load_ctx(b, c):
            # per-chunk bf16 context tile, staged through two f32 half-chunk DMAs
            xc = xbf_pool.tile([P, TPC, d], bf16, tag="xbf")
            r0 = b * NT + c * TPC
            for hf in range(2):
                cin = cin_pool.tile([P, 2, d], f32, tag="cin")
                nc.sync.dma_start(
                    out=cin,
                    in_=ctx3[r0 + 2 * hf : r0 + 2 * hf + 2].rearrange(
                        "n p dd -> p n dd"
                    ),
                )
                nc.vector.tensor_copy(out=xc[:, 2 * hf : 2 * hf + 2, :], in_=cin)
            xbf_tiles[(b, c)] = xc

        def emit_transpose(b, T):
            xt = xt_pool.tile([P, TPC, KO, P], bf16, tag="xt", name="xt")
            nc.sync.dma_start(out=xt, in_=xbf_tiles[(b, T)], transpose=True)
            xt_tiles[(b, T)] = xt

        # ---------------- interleaved preamble loads (sync ring) ----------------
        # w1 first chunk feeds the entity path; contexts(b0,c0) + w2 low
        # h-chunks feed the first main matmuls; the rest stream behind.
        load_w(w1_bf, w1_3d, 0)
        load_ctx(0, 0)
        nc.sync.dma_start(
            out=v_st, in_=ins["v_w"].rearrange("(ho p) o -> p ho o", p=P)
        )
        nc.vector.tensor_copy(out=v_bf, in_=v_st)
        load_w(w2_bf, w2_3d, 0)
        emit_transpose(0, 0)
        load_w(w2_bf, w2_3d, 1)
        load_ctx(0, 1)
        load_w(w2_bf, w2_3d, 2)
        load_w(w1_bf, w1_3d, 1)
        load_ctx(0, 2)
        load_w(w2_bf, w2_3d, 3)
        load_w(w1_bf, w1_3d, 2)
        load_w(w1_bf, w1_3d, 3)
        load_ctx(0, 3)

        # ---------------- entity path: reb[:, ho, b] = (entities@w1 + b1+b2)^T ----
        reb_sb = consts.tile([P, HO, b_loc], f32, tag="reb")
        with tc.tile_pool(name="ps_pre", bufs=2, space="PSUM") as ps_pre:
            entT_bf = consts.tile([P, KO, b_loc], bf16, tag="entT")
            for ko in range(KO):
                etr = ps_pre.tile([P, EP], f32, tag="pre")
                nc.tensor.transpose(etr, ent_sb[:, ko * P : (ko + 1) * P], id32)
                nc.vector.tensor_copy(out=entT_bf[:, ko, :], in_=etr[:, :b_loc])

            re_sb = consts.tile([EP, h], f32, tag="re_sb")
            nc.vector.memset(re_sb, 0.0)
            for q in range(NQ):
                re_ps = ps_pre.tile([b_loc, QW], f32, tag="re")
                for ko in range(KO):
                    nc.tensor.matmul(
                        re_ps,
                        lhsT=entT_bf[:, ko, :],
                        rhs=w1_bf[:, ko, q * QW : (q + 1) * QW],
                        start=(ko == 0),
                        stop=(ko == KO - 1),
                    )
                nc.scalar.copy(out=re_sb[:b_loc, q * QW : (q + 1) * QW], in_=re_ps)
                for ho in range(q * QW // P, (q + 1) * QW // P):
                    rtr = ps_pre.tile([P, EP], f32, tag="pre")
                    nc.tensor.transpose(rtr, re_sb[:, ho * P : (ho + 1) * P], id32)
                    nc.vector.tensor_scalar(
                        out=reb_sb[:, ho, :],
                        in0=rtr[:, :b_loc],
                        scalar1=bias_sb[:, ho : ho + 1],
                        scalar2=None,
                        op0=mybir.AluOpType.add,
                    )

        # ---------------- main-loop PSUM pools ----------------
        ps_rc = ctx.enter_context(tc.tile_pool(name="ps_rc", bufs=4, space="PSUM"))
        ps_sc = ctx.enter_context(tc.tile_pool(name="ps_sc", bufs=2, space="PSUM"))
        ps_wt = ctx.enter_context(tc.tile_pool(name="ps_wt", bufs=2, space="PSUM"))

        # per-batch state shared between emission helpers
        state = {}

        def emit_matvec(b, T, ho):
            st = state[b]
            if ho == 0:
                st["sc"][T] = ps_sc.tile([1, TCHUNK], f32, tag="sc", name="sc_ps")
            nc.tensor.matmul(
                st["sc"][T],
                lhsT=v_bf[:, ho, :],
                rhs=st["th"].pop((T, ho)),
                start=(ho == 0),
                stop=(ho == HO - 1),
            )

        def emit_score_copy(b, T):
            st = state[b]
            swb = sm_pool.tile([1, TCHUNK], f16, tag="swb", bufs=3)
            nc.scalar.copy(out=swb, in_=st["sc"][T])
            st["swb"][T] = swb

        def emit_score_transpose(b, T):
            st = state[b]
            swb = st["swb"].pop(T)
            for j in range(TPC):
                cidx = T * TPC + j
                nc.tensor.matmul(
                    st["wt"][:, cidx : cidx + 1],
                    lhsT=swb[:, j * P : (j + 1) * P],
                    rhs=ones1_f16,
                    start=(T == 0 and j == 0),
                    stop=(T == NC - 1 and j == TPC - 1),
                )

        def emit_softmax(b):
            st = state[b]
            wt = st["wt"]
            ew = sm_pool.tile([P, NT], f16, tag="ew")
            asum = sm_pool.tile([P, 1], f32, tag="asum")
            nc.scalar.activation(
                out=ew, in_=wt[:, :NT], func=AF.Exp, accum_out=asum
            )
            asum16 = sm_pool.tile([P, 1], f16, tag="asum16")
            nc.vector.tensor_copy(out=asum16, in_=asum)
            # cross-partition total, broadcast to every partition via ones^T @ asum
            nc.tensor.matmul(
                wt[:, NT : NT + 1], lhsT=ones128_f16, rhs=asum16, start=True, stop=True
            )
            rb = sm_pool.tile([P, 1], f32, tag="rb")
            nc.vector.reciprocal(out=rb, in_=wt[:, NT : NT + 1])
            wts = sm_pool.tile([P, NT], f32, tag="wts")
            nc.vector.tensor_scalar_mul(out=wts, in0=ew, scalar1=rb)
            st["wts"] = wts

        def emit_stage_f(b):
            st = state[b]
            wts = st["wts"]
            last = b == b_loc - 1
            for t in range(NT):
                xc = xbf_tiles[(b, t // TPC)]
                src = xc[:, t % TPC, :]
                ot = out_pool.tile([P, d], f32, tag="ot")
                if last and t % 2 == 1:
                    # split the tail multiplies across ACT and DVE
                    nc.scalar.activation(
                        out=ot, in_=src, func=AF.Copy, scale=wts[:, t : t + 1]
                    )
                else:
                    nc.vector.tensor_scalar_mul(
                        out=ot, in0=src, scalar1=wts[:, t : t + 1]
                    )
                # stores ride the ACT HWDGE ring mid-kernel (sync ring is busy
                # with transposes); the tail batch uses the now-idle sync ring.
                eng = nc.sync if last else nc.scalar
                eng.dma_start(out=out3[b * NT + t], in_=ot)
            for c in range(NC):
                xbf_tiles.pop((b, c))

        # ---------------- main loop over local batches ----------------
        for b in range(b_loc):
            state[b] = {"th": {}, "sc": {}, "swb": {}, "wt": None}
            state[b]["wt"] = ps_wt.tile([P, NT + 1], f32, tag="wt", name="wt_ps")

            for T in range(NC):
                # prefetch pipeline on the sync ring: load next batch's chunk T,
                # then queue the transpose for this batch's chunk T+1 (or the
                # next batch's chunk 0) so xt is always one chunk ahead.
                if b + 1 < b_loc:
                    load_ctx(b + 1, T)
                if T + 1 < NC:
                    emit_transpose(b, T + 1)
                elif b + 1 < b_loc:
                    emit_transpose(b + 1, 0)
                xt = xt_tiles.pop((b, T))

                for ho in range(HO):
                    rc = ps_rc.tile([P, TCHUNK], f32, tag="rc")
                    for ko in range(KO):
                        nc.tensor.matmul(
                            rc,
                            lhsT=w2_bf[:, ko, ho * P : (ho + 1) * P],
                            rhs=xt[:, :, ko, :],
                            start=(ko == 0),
                            stop=(ko == KO - 1),
                        )
                    th = th_pool.tile([P, TCHUNK], bf16, tag="th")
                    nc.scalar.activation(
                        out=th,
                        in_=rc,
                        func=AF.Tanh,
                        bias=reb_sb[:, ho, b : b + 1],
                        scale=1.0,
                    )
                    state[b]["th"][(T, ho)] = th

                    # deferred PE work, staggered so it never waits on ACT
                    if ho >= 1:
                        emit_matvec(b, T, ho - 1)
                    if T >= 1:
                        if ho == 0:
                            emit_matvec(b, T - 1, HO - 1)
                            emit_score_copy(b, T - 1)
                        elif ho == 1:
                            emit_score_transpose(b, T - 1)
                    elif b >= 1:
                        # previous batch's tail rides this batch's first chunk
                        if ho == 0:
                            emit_matvec(b - 1, NC - 1, HO - 1)
                            emit_score_copy(b - 1, NC - 1)
                        elif ho == 1:
                            emit_score_transpose(b - 1, NC - 1)
                        elif ho == 2:
                            emit_softmax(b - 1)
                        elif ho == 3:
                            emit_stage_f(b - 1)
                            del state[b - 1]

        # tail: last batch's remaining score work + softmax + stage F
        bl = b_loc - 1
        emit_matvec(bl, NC - 1, HO - 1)
        emit_score_copy(bl, NC - 1)
        emit_score_transpose(bl, NC - 1)
        emit_softmax(bl)
        emit_stage_f(bl)


def build_module(b_loc=B_LOC, s=S, d=D, h=H):
    """Build and compile the Bacc module for one core (SPMD-replicated)."""
    import concourse.mybir as mybir
    import concourse.tile as tile
    from concourse import bacc

    f32 = mybir.dt.float32
    nc = bacc.Bacc("TRN2", target_bir_lowering=False, debug=False)

    ins = {
        "contexts": nc.dram_tensor("contexts", [b_loc * s, d], f32, kind="ExternalInput").ap(),
        "entities": nc.dram_tensor("entities", [b_loc, d], f32, kind="ExternalInput").ap(),
        "w1_w": nc.dram_tensor("w1_w", [d, h], f32, kind="ExternalInput").ap(),
        "w2_w": nc.dram_tensor("w2_w", [d, h], f32, kind="ExternalInput").ap(),
        "w1_b": nc.dram_tensor("w1_b", [h], f32, kind="ExternalInput").ap(),
        "w2_b": nc.dram_tensor("w2_b", [h], f32, kind="ExternalInput").ap(),
        "v_w": nc.dram_tensor("v_w", [h, 1], f32, kind="ExternalInput").ap(),
    }
    out_ap = nc.dram_tensor("out", [b_loc * s, d], f32, kind="ExternalOutput").ap()

    with tile.TileContext(nc) as tc:
        build_attention(tc, out_ap, ins, b_loc=b_loc, s=s, d=d, h=h)

    nc.compile()
    return nc


_NC_CACHE = {}


def _get_module():
    key = (B_LOC, S, D, H)
    if key not in _NC_CACHE:
        _NC_CACHE[key] = build_module(*key)
    return _NC_CACHE[key]


def make_in_maps(inputs):
    entities = np.ascontiguousarray(np.asarray(inputs["entities"], np.float32))
    contexts = np.ascontiguousarray(np.asarray(inputs["contexts"], np.float32))
    shared = {
        k: np.ascontiguousarray(np.asarray(inputs[k], np.float32))
        for k in ("w1_w", "w2_w", "w1_b", "w2_b", "v_w")
    }
    in_maps = []
    for c in range(N_CORES):
        in_maps.append(
            dict(
                entities=entities[c * B_LOC : (c + 1) * B_LOC],
                contexts=contexts[c * B_LOC : (c + 1) * B_LOC].reshape(B_LOC * S, D),
                **shared,
            )
        )
    return in_maps


def run(inputs, trace=False, **kwargs):
    """Run on all 8 cores; returns (full_output, BassKernelResults)."""
    from concourse.bass_utils import run_bass_kernel_spmd

    nc = _get_module()
    res = run_bass_kernel_spmd(
        nc, make_in_maps(inputs), core_ids=list(range(N_CORES)), trace=trace, **kwargs
    )
    out = np.concatenate(
        [res.results[c]["out"].reshape(B_LOC, S, D) for c in range(N_CORES)], axis=0
    )
    return out, res


def kernel(**inputs) -> np.ndarray:
    out, _ = run(inputs, trace=False)
    return out


# revision 10
# speedup vs baseline: 1.2818x; 1.2623x over previous
"""Trainium2 Bass kernel for nn_AttentionLayer (additive attention pooling).

reference math:
    re = entities @ w1_w + w1_b                  # [B, H]
    rc = contexts @ w2_w + w2_b                  # [B, S, H]
    scores = tanh(re[:,None,:] + rc) @ v_w + v_b # [B, S, 1]
    weights = softmax(scores, axis=1)
    out = weights * contexts                     # [B, S, D]

Sharding: data-parallel over B across 8 cores (4 batches/core), weights
replicated.  Inside each core: bf16 TensorEngine matmuls (f32 accumulate),
softmax in f32/f16.  v_b is dropped (softmax is shift-invariant).

v2 dataflow (per core):
  - contexts/weights stream in via SWDGE (gpsimd) DMAs that cast f32->bf16
    in flight; interleaved so the first main matmul fires ~10us in.
  - per 512-token chunk: one xbar DMA transpose (sync ring) makes the
    d-major rhs; 64 accumulating bf16 matmuls produce rc; tanh(+re bias)
    on ACT; a v-matvec accumulates chunk scores in PSUM.
  - matvec for (chunk,ho) is emitted one ho-group later than its tanh so
    the PE never waits on ACT.
  - chunk scores [1,512] are copied to fp16 and transposed token-major via
    4 tiny PE matmuls into a per-batch [128,16] PSUM tile; softmax runs
    full-width (exp+accum on ACT, cross-partition total via a ones-matmul,
    reciprocal on DVE) - no single-lane work.
  - out tiles = bf16 contexts * per-token weight (DVE), stored via the
    scalar-engine HWDGE ring so stores never queue behind transposes.
"""

import sys

for _p in ("/opt/trn_rl_repo", "/root/.axon_site/_ro/trn_rl_repo"):
    if _p not in sys.path:
        sys.path.insert(0, _p)

import numpy as np

B, S, D, H = 32, 2048, 1024, 1024
N_CORES = 8
B_LOC = B // N_CORES          # batches per core
P = 128
TCHUNK = 512                  # tokens per chunk (moving free dim of main matmul)


def build_attention(tc, out_ap, ins, b_loc=B_LOC, s=S, d=D, h=H):
    """Emit the per-core kernel into TileContext `tc`.

    out_ap: DRAM AP [b_loc*s, d] f32
    ins: dict of DRAM APs: contexts [b_loc*s, d], entities [b_loc, d],
         w1_w [d, h], w2_w [d, h], w1_b [h], w2_b [h], v_w [h, 1]
    """
    from contextlib import ExitStack

    import concourse.mybir as mybir
    from concourse.masks import make_identity

    nc = tc.nc
    f32 = mybir.dt.float32
    bf16 = mybir.dt.bfloat16
    f16 = mybir.dt.float16
    AF = mybir.ActivationFunctionType

    KO = d // P                   # contraction k-tiles
    HO = h // P                   # h tiles
    NT = s // P                   # 128-token tiles per batch
    NC = s // TCHUNK              # chunks per batch
    TPC = TCHUNK // P             # token tiles per chunk
    QW = 256                      # h-chunk width for weight staging DMAs
    NQ = h // QW
    EP = 32                       # padded partition count for entity transposes
    assert d % P == 0 and h % P == 0 and s % TCHUNK == 0 and b_loc <= EP

    ctx3 = ins["contexts"].rearrange("(n p) dd -> n p dd", p=P)   # [b_loc*NT, P, d]
    out3 = out_ap.rearrange("(n p) dd -> n p dd", p=P)
    w1_3d = ins["w1_w"].rearrange("(ko p) hh -> p ko hh", p=P)
    w2_3d = ins["w2_w"].rearrange("(ko p) hh -> p ko hh", p=P)

    with ExitStack() as ctx:
        consts = ctx.enter_context(tc.tile_pool(name="consts", bufs=1))
        wpool = ctx.enter_context(tc.tile_pool(name="wpool", bufs=1))

        # ---------------- constants (tiny, sync ring) ----------------
        id32 = consts.tile([EP, EP], f32, tag="id32")
        make_identity(nc, id32)
        ones1_f16 = consts.tile([1, 1], f16, tag="ones1")
        nc.vector.memset(ones1_f16, 1.0)
        ones128_f16 = consts.tile([P, P], f16, tag="ones128")
        nc.vector.memset(ones128_f16, 1.0)

        ent_sb = consts.tile([EP, d], f32, tag="ent")
        nc.vector.memset(ent_sb, 0.0)
        nc.sync.dma_start(out=ent_sb[:b_loc, :], in_=ins["entities"][:, :])

        b1_sb = consts.tile([P, HO], f32, tag="b1")
        b2_sb = consts.tile([P, HO], f32, tag="b2")
        nc.sync.dma_start(out=b1_sb, in_=ins["w1_b"].rearrange("(ho p) -> p ho", p=P))
        nc.sync.dma_start(out=b2_sb, in_=ins["w2_b"].rearrange("(ho p) -> p ho", p=P))
        bias_sb = consts.tile([P, HO], f32, tag="bias")
        nc.vector.tensor_add(out=bias_sb, in0=b1_sb, in1=b2_sb)

        # ---------------- weight / context tiles ----------------
        w1_bf = wpool.tile([P, KO, h], bf16, tag="w1bf")
        w2_bf = wpool.tile([P, KO, h], bf16, tag="w2bf")
        v_bf = consts.tile([P, HO, 1], bf16, tag="v_bf")
        v_st = consts.tile([P, HO, 1], f32, tag="v_st")

        xbf_pool = ctx.enter_context(tc.tile_pool(name="xbf", bufs=8))
        xt_pool = ctx.enter_context(tc.tile_pool(name="xt", bufs=3))
        th_pool = ctx.enter_context(tc.tile_pool(name="th", bufs=4))
        out_pool = ctx.enter_context(tc.tile_pool(name="outp", bufs=6))
        sm_pool = ctx.enter_context(tc.tile_pool(name="smx", bufs=2))
        cin_pool = ctx.enter_context(tc.tile_pool(name="cin", bufs=3))
        wst_pool = ctx.enter_context(tc.tile_pool(name="wst", bufs=2))

        def load_w(dst3, src3, q):
            # HWDGE f32 load + DVE cast to bf16
            wst = wst_pool.tile([P, KO, QW], f32, tag="wst")
            nc.sync.dma_start(out=wst, in_=src3[:, :, q * QW : (q + 1) * QW])
            nc.vector.tensor_copy(out=dst3[:, :, q * QW : (q + 1) * QW], in_=wst)

        xbf_tiles = {}
        xt_tiles = {}

        def load_ctx(b, c):
            # per-chunk bf16 context tile, staged through two f32 half-chunk DMAs
            xc = xbf_pool.tile([P, TPC, d], bf16, tag="xbf")
            r0 = b * NT + c * TPC
            for hf in range(2):
                cin = cin_pool.tile([P, 2, d], f32, tag="cin")
                nc.sync.dma_start(
                    out=cin,
                    in_=ctx3[r0 + 2 * hf : r0 + 2 * hf + 2].rearrange(
                        "n p dd -> p n dd"
                    ),
                )
                nc.vector.tensor_copy(out=xc[:, 2 * hf : 2 * hf + 2, :], in_=cin)
            xbf_tiles[(b, c)] = xc

        def emit_transpose(b, T):
            xt = xt_pool.tile([P, TPC, KO, P], bf16, tag="xt", name="xt")
            nc.sync.dma_start(out=xt, in_=xbf_tiles[(b, T)], transpose=True)
            xt_tiles[(b, T)] = xt

        # ---------------- interleaved preamble loads (sync ring) ----------------
        # w1 first chunk feeds the entity path; contexts(b0,c0) + w2 low
        # h-chunks feed the first main matmuls; the rest stream behind.
        load_w(w1_bf, w1_3d, 0)
        load_ctx(0, 0)
        nc.sync.dma_start(
            out=v_st, in_=ins["v_w"].rearrange("(ho p) o -> p ho o", p=P)
        )
        nc.vector.tensor_copy(out=v_bf, in_=v_st)
        load_w(w2_bf, w2_3d, 0)
        emit_transpose(0, 0)
        load_w(w2_bf, w2_3d, 1)
        load_ctx(0, 1)
        load_w(w2_bf, w2_3d, 2)
        load_w(w1_bf, w1_3d, 1)
        load_ctx(0, 2)
        load_w(w2_bf, w2_3d, 3)
        load_w(w1_bf, w1_3d, 2)
        load_w(w1_bf, w1_3d, 3)
        load_ctx(0, 3)

        # ---------------- entity path: reb[:, ho, b] = (entities@w1 + b1+b2)^T ----
        reb_sb = consts.tile([P, HO, b_loc], f32, tag="reb")
        with tc.tile_pool(name="ps_pre", bufs=2, space="PSUM") as ps_pre:
            entT_bf = consts.tile([P, KO, b_loc], bf16, tag="entT")
            for ko in range(KO):
                etr = ps_pre.tile([P, EP], f32, tag="pre")
                nc.tensor.transpose(etr, ent_sb[:, ko * P : (ko + 1) * P], id32)
                nc.vector.tensor_copy(out=entT_bf[:, ko, :], in_=etr[:, :b_loc])

            re_sb = consts.tile([EP, h], f32, tag="re_sb")
            nc.vector.memset(re_sb, 0.0)
            for q in range(NQ):
                re_ps = ps_pre.tile([b_loc, QW], f32, tag="re")
                for ko in range(KO):
                    nc.tensor.matmul(
                        re_ps,
                        lhsT=entT_bf[:, ko, :],
                        rhs=w1_bf[:, ko, q * QW : (q + 1) * QW],
                        start=(ko == 0),
                        stop=(ko == KO - 1),
                    )
                nc.scalar.copy(out=re_sb[:b_loc, q * QW : (q + 1) * QW], in_=re_ps)
                for ho in range(q * QW // P, (q + 1) * QW // P):
                    rtr = ps_pre.tile([P, EP], f32, tag="pre")
                    nc.tensor.transpose(rtr, re_sb[:, ho * P : (ho + 1) * P], id32)
                    nc.vector.tensor_scalar(
                        out=reb_sb[:, ho, :],
                        in0=rtr[:, :b_loc],
                        scalar1=bias_sb[:, ho : ho + 1],
                        scalar2=None,
                        op0=mybir.AluOpType.add,
                    )

        # ---------------- main-loop PSUM pools ----------------
        ps_rc = ctx.enter_context(tc.tile_pool(name="ps_rc", bufs=4, space="PSUM"))
        ps_sc = ctx.enter_context(tc.tile_pool(name="ps_sc", bufs=2, space="PSUM"))
        ps_wt = ctx.enter_context(tc.tile_pool(name="ps_wt", bufs=2, space="PSUM"))

        # per-batch state shared between emission helpers
        state = {}

        def emit_matvec(b, T, ho):
            st = state[b]
            if ho == 0:
                st["sc"][T] = ps_sc.tile([1, TCHUNK], f32, tag="sc", name="sc_ps")
            nc.tensor.matmul(
                st["sc"][T],
                lhsT=v_bf[:, ho, :],
                rhs=st["th"].pop((T, ho)),
                start=(ho == 0),
                stop=(ho == HO - 1),
            )

        def emit_score_copy(b, T):
            st = state[b]
            swb = sm_pool.tile([1, TCHUNK], f16, tag="swb", bufs=3)
            nc.scalar.copy(out=swb, in_=st["sc"][T])
            st["swb"][T] = swb

        def emit_score_transpose(b, T):
            st = state[b]
            swb = st["swb"].pop(T)
            for j in range(TPC):
                cidx = T * TPC + j
                nc.tensor.matmul(
                    st["wt"][:, cidx : cidx + 1],
                    lhsT=swb[:, j * P : (j + 1) * P],
                    rhs=ones1_f16,
                    start=(T == 0 and j == 0),
                    stop=(T == NC - 1 and j == TPC - 1),
                )

        def emit_softmax(b):
            st = state[b]
            wt = st["wt"]
            ew = sm_pool.tile([P, NT], f16, tag="ew")
            asum = sm_pool.tile([P, 1], f32, tag="asum")
            nc.scalar.activation(
                out=ew, in_=wt[:, :NT], func=AF.Exp, accum_out=asum
            )
            asum16 = sm_pool.tile([P, 1], f16, tag="asum16")
            nc.vector.tensor_copy(out=asum16, in_=asum)
            # cross-partition total, broadcast to every partition via ones^T @ asum
            nc.tensor.matmul(
                wt[:, NT : NT + 1], lhsT=ones128_f16, rhs=asum16, start=True, stop=True
            )
            rb = sm_pool.tile([P, 1], f32, tag="rb")
            nc.vector.reciprocal(out=rb, in_=wt[:, NT : NT + 1])
            wts = sm_pool.tile([P, NT], f32, tag="wts")
            nc.vector.tensor_scalar_mul(out=wts, in0=ew, scalar1=rb)
            st["wts"] = wts

        def emit_stage_f(b, c):
            # one chunk's worth (4 tiles) of out = weight * contexts; stores
            # ride the otherwise-idle SWDGE (gpsimd) ring so they never
            # FIFO-block tanhs (ACT) or casts (DVE).
            st = state[b]
            wts = st["wts"]
            last = b == b_loc - 1
            for t in range(c * TPC, (c + 1) * TPC):
                src = xbf_tiles[(b, c)][:, t % TPC, :]
                ot = out_pool.tile([P, d], f32, tag="ot")
                if last and t % 2 == 1:
                    # split the tail multiplies across ACT and DVE
                    nc.scalar.activation(
                        out=ot, in_=src, func=AF.Copy, scale=wts[:, t : t + 1]
                    )
                else:
                    nc.vector.tensor_scalar_mul(
                        out=ot, in0=src, scalar1=wts[:, t : t + 1]
                    )
                nc.gpsimd.dma_start(out=out3[b * NT + t], in_=ot)
            xbf_tiles.pop((b, c))

        # ---------------- main loop over local batches ----------------
        for b in range(b_loc):
            state[b] = {"th": {}, "sc": {}, "swb": {}, "wt": None}
            state[b]["wt"] = ps_wt.tile([P, NT + 1], f32, tag="wt", name="wt_ps")

            for T in range(NC):
                # prefetch pipeline on the sync ring: load next batch's chunk T,
                # then queue the transpose for this batch's chunk T+1 (or the
                # next batch's chunk 0) so xt is always one chunk ahead.
                if b + 1 < b_loc:
                    load_ctx(b + 1, T)
                if T + 1 < NC:
                    emit_transpose(b, T + 1)
                elif b + 1 < b_loc:
                    emit_transpose(b + 1, 0)
                xt = xt_tiles.pop((b, T))

                for ho in range(HO):
                    rc = ps_rc.tile([P, TCHUNK], f32, tag="rc")
                    for ko in range(KO):
                        nc.tensor.matmul(
                            rc,
                            lhsT=w2_bf[:, ko, ho * P : (ho + 1) * P],
                            rhs=xt[:, :, ko, :],
                            start=(ko == 0),
                            stop=(ko == KO - 1),
                        )
                    th = th_pool.tile([P, TCHUNK], bf16, tag="th")
                    nc.scalar.activation(
                        out=th,
                        in_=rc,
                        func=AF.Tanh,
                        bias=reb_sb[:, ho, b : b + 1],
                        scale=1.0,
                    )
                    state[b]["th"][(T, ho)] = th

                    # deferred PE work, staggered so it never waits on ACT
                    if ho >= 1:
                        emit_matvec(b, T, ho - 1)
                    if T >= 1:
                        if ho == 0:
                            emit_matvec(b, T - 1, HO - 1)
                            emit_score_copy(b, T - 1)
                        elif ho == 1:
                            emit_score_transpose(b, T - 1)
                    elif b >= 1:
                        # previous batch's tail rides this batch's first chunk
                        if ho == 0:
                            emit_matvec(b - 1, NC - 1, HO - 1)
                            emit_score_copy(b - 1, NC - 1)
                        elif ho == 1:
                            emit_score_transpose(b - 1, NC - 1)
                        elif ho == 2:
                            emit_softmax(b - 1)
                    if b >= 1 and ho == 3:
                        # spread prev batch's stage F across this batch's chunks
                        emit_stage_f(b - 1, T)
                        if T == NC - 1:
                            del state[b - 1]

        # tail: last batch's remaining score work + softmax + stage F
        bl = b_loc - 1
        emit_matvec(bl, NC - 1, HO - 1)
        emit_score_copy(bl, NC - 1)
        emit_score_transpose(bl, NC - 1)
        emit_softmax(bl)
        for c in range(NC):
            emit_stage_f(bl, c)


def build_module(b_loc=B_LOC, s=S, d=D, h=H):
    """Build and compile the Bacc module for one core (SPMD-replicated)."""
    import concourse.mybir as mybir
    import concourse.tile as tile
    from concourse import bacc

    f32 = mybir.dt.float32
    nc = bacc.Bacc("TRN2", target_bir_lowering=False, debug=False)

    ins = {
        "contexts": nc.dram_tensor("contexts", [b_loc * s, d], f32, kind="ExternalInput").ap(),
        "entities": nc.dram_tensor("entities", [b_loc, d], f32, kind="ExternalInput").ap(),
        "w1_w": nc.dram_tensor("w1_w", [d, h], f32, kind="ExternalInput").ap(),
        "w2_w": nc.dram_tensor("w2_w", [d, h], f32, kind="ExternalInput").ap(),
        "w1_b": nc.dram_tensor("w1_b", [h], f32, kind="ExternalInput").ap(),
        "w2_b": nc.dram_tensor("w2_b", [h], f32, kind="ExternalInput").ap(),
        "v_w": nc.dram_tensor("v_w", [h, 1], f32, kind="ExternalInput").ap(),
    }
    out_ap = nc.dram_tensor("out", [b_loc * s, d], f32, kind="ExternalOutput").ap()

    with tile.TileContext(nc) as tc:
        build_attention(tc, out_ap, ins, b_loc=b_loc, s=s, d=d, h=h)

    nc.compile()
    return nc


_NC_CACHE = {}


def _get_module():
    key = (B_LOC, S, D, H)
    if key not in _NC_CACHE:
        _NC_CACHE[key] = build_module(*key)
    return _NC_CACHE[key]


def make_in_maps(inputs):
    entities = np.ascontiguousarray(np.asarray(inputs["entities"], np.float32))
    contexts = np.ascontiguousarray(np.asarray(inputs["contexts"], np.float32))
    shared = {
        k: np.ascontiguousarray(np.asarray(inputs[k], np.float32))
        for k in ("w1_w", "w2_w", "w1_b", "w2_b", "v_w")
    }
    in_maps = []
    for c in range(N_CORES):
        in_maps.append(
            dict(
                entities=entities[c * B_LOC : (c + 1) * B_LOC],
                contexts=contexts[c * B_LOC : (c + 1) * B_LOC].reshape(B_LOC * S, D),
                **shared,
            )
        )
    return in_maps


def run(inputs, trace=False, **kwargs):
    """Run on all 8 cores; returns (full_output, BassKernelResults)."""
    from concourse.bass_utils import run_bass_kernel_spmd

    nc = _get_module()
    res = run_bass_kernel_spmd(
        nc, make_in_maps(inputs), core_ids=list(range(N_CORES)), trace=trace, **kwargs
    )
    out = np.concatenate(
        [res.results[c]["out"].reshape(B_LOC, S, D) for c in range(N_CORES)], axis=0
    )
    return out, res


def kernel(**inputs) -> np.ndarray:
    out, _ = run(inputs, trace=False)
    return out


# revision 15
# speedup vs baseline: 1.2838x; 1.0016x over previous
"""Trainium2 Bass kernel for nn_AttentionLayer (additive attention pooling).

reference math:
    re = entities @ w1_w + w1_b                  # [B, H]
    rc = contexts @ w2_w + w2_b                  # [B, S, H]
    scores = tanh(re[:,None,:] + rc) @ v_w + v_b # [B, S, 1]
    weights = softmax(scores, axis=1)
    out = weights * contexts                     # [B, S, D]

Sharding: data-parallel over B across 8 cores (4 batches/core), weights
replicated.  Inside each core: bf16 TensorEngine matmuls (f32 accumulate),
softmax in f32/f16.  v_b is dropped (softmax is shift-invariant).

v2 dataflow (per core):
  - contexts/weights stream in via SWDGE (gpsimd) DMAs that cast f32->bf16
    in flight; interleaved so the first main matmul fires ~10us in.
  - per 512-token chunk: one xbar DMA transpose (sync ring) makes the
    d-major rhs; 64 accumulating bf16 matmuls produce rc; tanh(+re bias)
    on ACT; a v-matvec accumulates chunk scores in PSUM.
  - matvec for (chunk,ho) is emitted one ho-group later than its tanh so
    the PE never waits on ACT.
  - chunk scores [1,512] are copied to fp16 and transposed token-major via
    4 tiny PE matmuls into a per-batch [128,16] PSUM tile; softmax runs
    full-width (exp+accum on ACT, cross-partition total via a ones-matmul,
    reciprocal on DVE) - no single-lane work.
  - out tiles = bf16 contexts * per-token weight (DVE), stored via the
    scalar-engine HWDGE ring so stores never queue behind transposes.
"""

import sys

for _p in ("/opt/trn_rl_repo", "/root/.axon_site/_ro/trn_rl_repo"):
    if _p not in sys.path:
        sys.path.insert(0, _p)

import numpy as np

B, S, D, H = 32, 2048, 1024, 1024
N_CORES = 8
B_LOC = B // N_CORES          # batches per core
P = 128
TCHUNK = 512                  # tokens per chunk (moving free dim of main matmul)


def build_attention(tc, out_ap, ins, b_loc=B_LOC, s=S, d=D, h=H):
    """Emit the per-core kernel into TileContext `tc`.

    out_ap: DRAM AP [b_loc*s, d] f32
    ins: dict of DRAM APs: contexts [b_loc*s, d], entities [b_loc, d],
         w1_w [d, h], w2_w [d, h], w1_b [h], w2_b [h], v_w [h, 1]
    """
    from contextlib import ExitStack

    import concourse.mybir as mybir
    from concourse.masks import make_identity

    nc = tc.nc
    f32 = mybir.dt.float32
    bf16 = mybir.dt.bfloat16
    f16 = mybir.dt.float16
    AF = mybir.ActivationFunctionType

    KO = d // P                   # contraction k-tiles
    HO = h // P                   # h tiles
    NT = s // P                   # 128-token tiles per batch
    NC = s // TCHUNK              # chunks per batch
    TPC = TCHUNK // P             # token tiles per chunk
    QW = 256                      # h-chunk width for weight staging DMAs
    NQ = h // QW
    EP = 32                       # padded partition count for entity transposes
    assert d % P == 0 and h % P == 0 and s % TCHUNK == 0 and b_loc <= EP

    ctx3 = ins["contexts"].rearrange("(n p) dd -> n p dd", p=P)   # [b_loc*NT, P, d]
    out3 = out_ap.rearrange("(n p) dd -> n p dd", p=P)
    w1_3d = ins["w1_w"].rearrange("(ko p) hh -> p ko hh", p=P)
    w2_3d = ins["w2_w"].rearrange("(ko p) hh -> p ko hh", p=P)

    with ExitStack() as ctx:
        consts = ctx.enter_context(tc.tile_pool(name="consts", bufs=1))
        wpool = ctx.enter_context(tc.tile_pool(name="wpool", bufs=1))

        # ---------------- constants (tiny, sync ring) ----------------
        id32 = consts.tile([EP, EP], f32, tag="id32")
        make_identity(nc, id32)
        ones1_f16 = consts.tile([1, 1], f16, tag="ones1")
        nc.vector.memset(ones1_f16, 1.0)
        ones128_f16 = consts.tile([P, P], f16, tag="ones128")
        nc.vector.memset(ones128_f16, 1.0)

        ent_sb = consts.tile([EP, d], f32, tag="ent")
        nc.vector.memset(ent_sb, 0.0)
        nc.sync.dma_start(out=ent_sb[:b_loc, :], in_=ins["entities"][:, :])

        b1_sb = consts.tile([P, HO], f32, tag="b1")
        b2_sb = consts.tile([P, HO], f32, tag="b2")
        nc.sync.dma_start(out=b1_sb, in_=ins["w1_b"].rearrange("(ho p) -> p ho", p=P))
        nc.sync.dma_start(out=b2_sb, in_=ins["w2_b"].rearrange("(ho p) -> p ho", p=P))
        bias_sb = consts.tile([P, HO], f32, tag="bias")
        nc.vector.tensor_add(out=bias_sb, in0=b1_sb, in1=b2_sb)

        # ---------------- weight / context tiles ----------------
        w1_bf = wpool.tile([P, KO, h], bf16, tag="w1bf")
        w2_bf = wpool.tile([P, KO, h], bf16, tag="w2bf")
        v_bf = consts.tile([P, HO, 1], bf16, tag="v_bf")
        v_st = consts.tile([P, HO, 1], f32, tag="v_st")

        xbf_pool = ctx.enter_context(tc.tile_pool(name="xbf", bufs=8))
        xt_pool = ctx.enter_context(tc.tile_pool(name="xt", bufs=3))
        th_pool = ctx.enter_context(tc.tile_pool(name="th", bufs=11))
        out_pool = ctx.enter_context(tc.tile_pool(name="outp", bufs=6))
        sm_pool = ctx.enter_context(tc.tile_pool(name="smx", bufs=2))
        cin_pool = ctx.enter_context(tc.tile_pool(name="cin", bufs=3))
        wst_pool = ctx.enter_context(tc.tile_pool(name="wst", bufs=2))

        def load_w(dst3, src3, q):
            # HWDGE f32 load + DVE cast to bf16
            wst = wst_pool.tile([P, KO, QW], f32, tag="wst")
            nc.sync.dma_start(out=wst, in_=src3[:, :, q * QW : (q + 1) * QW])
            nc.vector.tensor_copy(out=dst3[:, :, q * QW : (q + 1) * QW], in_=wst)

        xbf_tiles = {}
        xt_tiles = {}

        def load_ctx(b, c):
            # per-chunk bf16 context tile, staged through two f32 half-chunk DMAs
            xc = xbf_pool.tile([P, TPC, d], bf16, tag="xbf")
            r0 = b * NT + c * TPC
            for hf in range(2):
                cin = cin_pool.tile([P, 2, d], f32, tag="cin")
                nc.sync.dma_start(
                    out=cin,
                    in_=ctx3[r0 + 2 * hf : r0 + 2 * hf + 2].rearrange(
                        "n p dd -> p n dd"
                    ),
                )
                nc.vector.tensor_copy(out=xc[:, 2 * hf : 2 * hf + 2, :], in_=cin)
            xbf_tiles[(b, c)] = xc

        def emit_transpose(b, T):
            xt = xt_pool.tile([P, TPC, KO, P], bf16, tag="xt", name="xt")
            nc.sync.dma_start(out=xt, in_=xbf_tiles[(b, T)], transpose=True)
            xt_tiles[(b, T)] = xt

        # ---------------- preamble loads (sync ring) ----------------
        # w1 streams first (the entity path sits at the head of the PE
        # program), then contexts(b0,c0) + w2 for the first main matmuls,
        # then the rest of batch 0.
        nc.sync.dma_start(
            out=v_st, in_=ins["v_w"].rearrange("(ho p) o -> p ho o", p=P)
        )
        nc.vector.tensor_copy(out=v_bf, in_=v_st)
        for q in range(NQ):
            load_w(w1_bf, w1_3d, q)
        load_ctx(0, 0)
        for q in range(NQ):
            load_w(w2_bf, w2_3d, q)
        emit_transpose(0, 0)
        load_ctx(0, 1)
        load_ctx(0, 2)
        load_ctx(0, 3)

        # ---------------- entity path: reb[:, ho, b] = (entities@w1 + b1+b2)^T ----
        reb_sb = consts.tile([P, HO, b_loc], f32, tag="reb")
        with tc.tile_pool(name="ps_pre", bufs=2, space="PSUM") as ps_pre:
            entT_bf = consts.tile([P, KO, b_loc], bf16, tag="entT")
            for ko in range(KO):
                etr = ps_pre.tile([P, EP], f32, tag="pre")
                nc.tensor.transpose(etr, ent_sb[:, ko * P : (ko + 1) * P], id32)
                nc.vector.tensor_copy(out=entT_bf[:, ko, :], in_=etr[:, :b_loc])

            re_sb = consts.tile([EP, h], f32, tag="re_sb")
            nc.vector.memset(re_sb, 0.0)
            for q in range(NQ):
                re_ps = ps_pre.tile([b_loc, QW], f32, tag="re")
                for ko in range(KO):
                    nc.tensor.matmul(
                        re_ps,
                        lhsT=entT_bf[:, ko, :],
                        rhs=w1_bf[:, ko, q * QW : (q + 1) * QW],
                        start=(ko == 0),
                        stop=(ko == KO - 1),
                    )
                nc.scalar.copy(out=re_sb[:b_loc, q * QW : (q + 1) * QW], in_=re_ps)
                for ho in range(q * QW // P, (q + 1) * QW // P):
                    rtr = ps_pre.tile([P, EP], f32, tag="pre")
                    nc.tensor.transpose(rtr, re_sb[:, ho * P : (ho + 1) * P], id32)
                    nc.vector.tensor_scalar(
                        out=reb_sb[:, ho, :],
                        in0=rtr[:, :b_loc],
                        scalar1=bias_sb[:, ho : ho + 1],
                        scalar2=None,
                        op0=mybir.AluOpType.add,
                    )

        # ---------------- main-loop PSUM pools ----------------
        ps_rc = ctx.enter_context(tc.tile_pool(name="ps_rc", bufs=4, space="PSUM"))
        ps_sc = ctx.enter_context(tc.tile_pool(name="ps_sc", bufs=2, space="PSUM"))
        ps_wt = ctx.enter_context(tc.tile_pool(name="ps_wt", bufs=2, space="PSUM"))

        # per-batch state shared between emission helpers
        state = {}

        def emit_matvec_block(b, T):
            # all 8 matvecs of a chunk back-to-back: entering/leaving the
            # matvec disturbs the main-MM LDW pipeline (~190ns bubble), so
            # pay it once per chunk rather than once per ho
            st = state[b]
            st["sc"][T] = ps_sc.tile([1, TCHUNK], f32, tag="sc", name="sc_ps")
            for ho in range(HO):
                nc.tensor.matmul(
                    st["sc"][T],
                    lhsT=v_bf[:, ho, :],
                    rhs=st["th"].pop((T, ho)),
                    start=(ho == 0),
                    stop=(ho == HO - 1),
                )

        def emit_score_copy(b, T):
            st = state[b]
            swb = sm_pool.tile([1, TCHUNK], f16, tag="swb", bufs=3)
            nc.scalar.copy(out=swb, in_=st["sc"][T])
            st["swb"][T] = swb

        def emit_score_transpose(b, T):
            st = state[b]
            swb = st["swb"].pop(T)
            for j in range(TPC):
                cidx = T * TPC + j
                nc.tensor.matmul(
                    st["wt"][:, cidx : cidx + 1],
                    lhsT=swb[:, j * P : (j + 1) * P],
                    rhs=ones1_f16,
                    start=(T == 0 and j == 0),
                    stop=(T == NC - 1 and j == TPC - 1),
                )

        def emit_softmax(b):
            st = state[b]
            wt = st["wt"]
            ew = sm_pool.tile([P, NT], f16, tag="ew")
            asum = sm_pool.tile([P, 1], f32, tag="asum")
            nc.scalar.activation(
                out=ew, in_=wt[:, :NT], func=AF.Exp, accum_out=asum
            )
            asum16 = sm_pool.tile([P, 1], f16, tag="asum16")
            nc.vector.tensor_copy(out=asum16, in_=asum)
            # cross-partition total, broadcast to every partition via ones^T @ asum
            nc.tensor.matmul(
                wt[:, NT : NT + 1], lhsT=ones128_f16, rhs=asum16, start=True, stop=True
            )
            rb = sm_pool.tile([P, 1], f32, tag="rb")
            nc.vector.reciprocal(out=rb, in_=wt[:, NT : NT + 1])
            wts = sm_pool.tile([P, NT], f32, tag="wts")
            nc.vector.tensor_scalar_mul(out=wts, in0=ew, scalar1=rb)
            st["wts"] = wts

        def emit_stage_f(b, c):
            # one chunk's worth (4 tiles) of out = weight * contexts; stores
            # ride the otherwise-idle SWDGE (gpsimd) ring so they never
            # FIFO-block tanhs (ACT) or casts (DVE).
            st = state[b]
            wts = st["wts"]
            last = b == b_loc - 1
            for t in range(c * TPC, (c + 1) * TPC):
                src = xbf_tiles[(b, c)][:, t % TPC, :]
                ot = out_pool.tile([P, d], f32, tag="ot")
                if last and t % 2 == 1:
                    # split the tail multiplies across ACT and DVE
                    nc.scalar.activation(
                        out=ot, in_=src, func=AF.Copy, scale=wts[:, t : t + 1]
                    )
                else:
                    nc.vector.tensor_scalar_mul(
                        out=ot, in0=src, scalar1=wts[:, t : t + 1]
                    )
                nc.gpsimd.dma_start(out=out3[b * NT + t], in_=ot)
            xbf_tiles.pop((b, c))

        # ---------------- main loop over local batches ----------------
        for b in range(b_loc):
            state[b] = {"th": {}, "sc": {}, "swb": {}, "wt": None}
            state[b]["wt"] = ps_wt.tile([P, NT + 1], f32, tag="wt", name="wt_ps")

            for T in range(NC):
                # prefetch pipeline on the sync ring: load next batch's chunk T,
                # then queue the transpose for this batch's chunk T+1 (or the
                # next batch's chunk 0) so xt is always one chunk ahead.
                if b + 1 < b_loc:
                    load_ctx(b + 1, T)
                if T + 1 < NC:
                    emit_transpose(b, T + 1)
                elif b + 1 < b_loc:
                    emit_transpose(b + 1, 0)
                xt = xt_tiles.pop((b, T))

                for ho in range(HO):
                    rc = ps_rc.tile([P, TCHUNK], f32, tag="rc")
                    for ko in range(KO):
                        nc.tensor.matmul(
                            rc,
                            lhsT=w2_bf[:, ko, ho * P : (ho + 1) * P],
                            rhs=xt[:, :, ko, :],
                            start=(ko == 0),
                            stop=(ko == KO - 1),
                        )
                    th = th_pool.tile([P, TCHUNK], bf16, tag="th")
                    nc.scalar.activation(
                        out=th,
                        in_=rc,
                        func=AF.Tanh,
                        bias=reb_sb[:, ho, b : b + 1],
                        scale=1.0,
                    )
                    state[b]["th"][(T, ho)] = th

                    # deferred PE work, staggered one chunk so it never
                    # waits on ACT
                    if T >= 1:
                        if ho == 0:
                            emit_matvec_block(b, T - 1)
                            emit_score_copy(b, T - 1)
                        elif ho == 1:
                            emit_score_transpose(b, T - 1)
                    elif b >= 1:
                        # previous batch's tail rides this batch's first chunk
                        if ho == 0:
                            emit_matvec_block(b - 1, NC - 1)
                            emit_score_copy(b - 1, NC - 1)
                        elif ho == 1:
                            emit_score_transpose(b - 1, NC - 1)
                        elif ho == 2:
                            emit_softmax(b - 1)
                    if b >= 1 and ho == 3:
                        # spread prev batch's stage F across this batch's chunks
                        emit_stage_f(b - 1, T)
                        if T == NC - 1:
                            del state[b - 1]

        # tail: last batch's remaining score work + softmax + stage F
        bl = b_loc - 1
        emit_matvec_block(bl, NC - 1)
        emit_score_copy(bl, NC - 1)
        emit_score_transpose(bl, NC - 1)
        emit_softmax(bl)
        for c in range(NC):
            emit_stage_f(bl, c)


def build_module(b_loc=B_LOC, s=S, d=D, h=H):
    """Build and compile the Bacc module for one core (SPMD-replicated)."""
    import concourse.mybir as mybir
    import concourse.tile as tile
    from concourse import bacc

    f32 = mybir.dt.float32
    nc = bacc.Bacc("TRN2", target_bir_lowering=False, debug=False)

    ins = {
        "contexts": nc.dram_tensor("contexts", [b_loc * s, d], f32, kind="ExternalInput").ap(),
        "entities": nc.dram_tensor("entities", [b_loc, d], f32, kind="ExternalInput").ap(),
        "w1_w": nc.dram_tensor("w1_w", [d, h], f32, kind="ExternalInput").ap(),
        "w2_w": nc.dram_tensor("w2_w", [d, h], f32, kind="ExternalInput").ap(),
        "w1_b": nc.dram_tensor("w1_b", [h], f32, kind="ExternalInput").ap(),
        "w2_b": nc.dram_tensor("w2_b", [h], f32, kind="ExternalInput").ap(),
        "v_w": nc.dram_tensor("v_w", [h, 1], f32, kind="ExternalInput").ap(),
    }
    out_ap = nc.dram_tensor("out", [b_loc * s, d], f32, kind="ExternalOutput").ap()

    with tile.TileContext(nc) as tc:
        build_attention(tc, out_ap, ins, b_loc=b_loc, s=s, d=d, h=h)

    nc.compile()
    return nc


_NC_CACHE = {}


def _get_module():
    key = (B_LOC, S, D, H)
    if key not in _NC_CACHE:
        _NC_CACHE[key] = build_module(*key)
    return _NC_CACHE[key]


def make_in_maps(inputs):
    entities = np.ascontiguousarray(np.asarray(inputs["entities"], np.float32))
    contexts = np.ascontiguousarray(np.asarray(inputs["contexts"], np.float32))
    shared = {
        k: np.ascontiguousarray(np.asarray(inputs[k], np.float32))
        for k in ("w1_w", "w2_w", "w1_b", "w2_b", "v_w")
    }
    in_maps = []
    for c in range(N_CORES):
        in_maps.append(
            dict(
                entities=entities[c * B_LOC : (c + 1) * B_LOC],
                contexts=contexts[c * B_LOC : (c + 1) * B_LOC].reshape(B_LOC * S, D),
                **shared,
            )
        )
    return in_maps


def run(inputs, trace=False, **kwargs):
    """Run on all 8 cores; returns (full_output, BassKernelResults)."""
    from concourse.bass_utils import run_bass_kernel_spmd

    nc = _get_module()
    res = run_bass_kernel_spmd(
        nc, make_in_maps(inputs), core_ids=list(range(N_CORES)), trace=trace, **kwargs
    )
    out = np.concatenate(
        [res.results[c]["out"].reshape(B_LOC, S, D) for c in range(N_CORES)], axis=0
    )
    return out, res


def kernel(**inputs) -> np.ndarray:
    out, _ = run(inputs, trace=False)
    return out
